# revision 139
# baseline (speedup 1.0000x reference)
import numpy as np
import ml_dtypes

import concourse.bass as bass
import concourse.bacc as bacc
import concourse.mybir as mybir
import concourse.tile as tile
from concourse import library_config
from concourse.bass_utils import run_bass_kernel_spmd
from concourse.masks import make_identity

NC = 8
CH = 128
NBUCK = 4
CPB = NC // NBUCK
TILE_COLS = 80
GCAP = 16
GCAP2 = 24
SUPER_COLS = 1
SUPER_COLS2 = 1
SCMERGE = 48
SCMERGE1 = 24
SCAT_LAG = 2
DEG_CAP = 1024
MMG = 10
NS = 10


def _split_multi_waits(nc, max_waits=1):
    n = 0
    for f in nc.m.functions:
        for bb in f.blocks:
            new_insts = []
            for inst in bb.instructions:
                si = inst.sync_info
                if si is not None and si.on_wait and len(si.on_wait) > max_waits:
                    waits = list(si.on_wait)
                    for w in waits[max_waits:]:
                        n += 1
                        new_insts.append(mybir.InstNoOp(
                            name=f"WSPLIT-{n}-{inst.name}",
                            sync_info=mybir.SyncInfo(on_wait=[w], on_update=[]),
                            bass_nofuse=True,
                            engine=inst.engine,
                        ))
                    si.on_wait = waits[:max_waits]
                new_insts.append(inst)
            bb.instructions.clear()
            for i in new_insts:
                bb.add_instruction(i)
    return n


def _wrap_idx(lst):
    w = lst.reshape(-1, 16).T
    return np.ascontiguousarray(np.tile(w, (8, 1)))


def _cumcount(key, n):
    o = np.argsort(key, kind="stable")
    sk = key[o]
    starts = np.flatnonzero(np.r_[True, sk[1:] != sk[:-1]])
    lens = np.diff(np.r_[starts, n])
    kk = np.arange(n, dtype=np.int64) - np.repeat(starts, lens)
    k = np.empty(n, dtype=np.int64)
    k[o] = kk
    return k


def _runs_capped(K, cap, gcap=None):
    groups = []
    q, n = 0, len(K)
    while q < n:
        k = int(K[q])
        if k == 0:
            q += 1
            continue
        g = 1
        lim = max(1, cap // k)
        if gcap is not None:
            lim = min(lim, gcap)
        while q + g < n and int(K[q + g]) == k and g < lim:
            g += 1
        groups.append((q, g, k))
        q += g
    return groups


def _prep(x, edge_index, edge_weights, W1, b1, W2, b2):
    N, DIN = x.shape
    DH = W1.shape[0]
    DOUT = W2.shape[0]
    E = edge_index.shape[1]
    assert DIN == DH, "partial buffers assume DIN == DH"
    per_core = -(-N // NC)
    SHARD = -(-per_core // (4 * CH)) * (4 * CH)
    NTAB = NC * SHARD
    BUCK = NTAB // NBUCK
    NCHUNK = SHARD // CH
    assert BUCK <= 32767 and NC % NBUCK == 0

    src = np.asarray(edge_index[0], dtype=np.int64)
    dst = np.asarray(edge_index[1], dtype=np.int64)
    w = np.asarray(edge_weights, dtype=np.float32)

    indeg = np.bincount(dst, minlength=N)
    order = np.argsort(-indeg, kind="stable")
    i_of = np.empty(N, dtype=np.int64)
    i_of[order] = np.arange(N)
    tpos = (i_of % NC) * SHARD + (i_of // NC)

    x_perm = np.zeros((NTAB, DIN), dtype=np.float32)
    x_perm[tpos] = np.asarray(x, dtype=np.float32)
    x_perm = x_perm.astype(ml_dtypes.bfloat16)
    x_own_pm = (x_perm.reshape(NC, NCHUNK, CH, DIN).transpose(0, 2, 1, 3)
                .reshape(NC, CH, NCHUNK * DIN))

    tsrc = tpos[src]
    tdst = tpos[dst]
    ec = tdst // SHARD
    ep = tdst % SHARD
    eb = tsrc // BUCK
    eloc = (tsrc % BUCK).astype(np.int16)
    c2 = tsrc // SHARD
    ps_ = (tsrc % SHARD) % CH
    qs_ = (tsrc % SHARD) // CH
    eloc2 = (ps_ * NCHUNK + qs_).astype(np.int16)

    cb = ec * NBUCK + eb
    cnt = np.bincount(cb * SHARD + ep, minlength=NC * NBUCK * SHARD)
    cnt = cnt.reshape(NC, NBUCK, SHARD)
    sq = (np.arange(SHARD) // CH) % 2
    par_regions = [np.flatnonzero(sq == p_) for p_ in (0, 1)]
    pi = np.concatenate(
        [reg[np.argsort(-cnt[:, :, reg], axis=2, kind="stable")]
         for reg in par_regions], axis=2)
    inv = np.empty_like(pi)
    np.put_along_axis(inv, pi, np.broadcast_to(np.arange(SHARD), pi.shape).copy(), axis=2)
    cnt_sorted = np.take_along_axis(cnt, pi, axis=2)
    Kb = np.maximum(cnt_sorted[:, :, ::CH].max(axis=0), 1)
    assert int(Kb.max()) <= TILE_COLS
    colsb = Kb.sum(axis=1)
    bucket_base = np.concatenate([[0], np.cumsum(colsb)]).astype(np.int64)
    TCOLS = int(bucket_base[-1])
    CB_MAX = int(colsb.max())
    coloff = np.zeros((NBUCK, NCHUNK), dtype=np.int64)
    for b in range(NBUCK):
        coloff[b] = bucket_base[b] + np.r_[0, np.cumsum(Kb[b])[:-1]]

    rp = inv[ec, eb, ep]
    k = _cumcount(cb * SHARD + rp, E)
    col = coloff[eb, rp // CH] + k
    slot = col * CH + (rp % CH)
    ell_idx = np.zeros((NC, TCOLS * CH), dtype=np.int16)
    ell_w = np.zeros((NC, TCOLS * CH), dtype=np.float32)
    ell_idx[ec, slot] = eloc
    ell_w[ec, slot] = w

    ell_idx_w = np.stack([_wrap_idx(ell_idx[c]) for c in range(NC)])
    ell_w_sb = np.ascontiguousarray(
        ell_w.reshape(NC, TCOLS, CH).transpose(0, 2, 1))

    def _supers(runs, cap):
        supers = []
        cur, cur_cols, c0 = [], 0, None
        for (q0, g, K, cs) in runs:
            cols = g * K
            if cur and cur_cols + cols > cap:
                supers.append((c0, cur_cols, tuple(cur)))
                cur, cur_cols = [], 0
            if not cur:
                c0 = cs
            cur.append((q0, g, K, cs))
            cur_cols += cols
        if cur:
            supers.append((c0, cur_cols, tuple(cur)))
        return supers

    agg_groups = [[] for _ in range(NBUCK)]
    HC = NCHUNK // 2
    for b in range(NBUCK):
        runs = []
        for p_ in range(2):
            runs += [(q0 + p_ * HC, g, K, int(coloff[b, q0 + p_ * HC]))
                     for (q0, g, K) in _runs_capped(
                         Kb[b][p_ * HC:(p_ + 1) * HC], TILE_COLS, GCAP)]
        agg_groups[b] = _supers(runs, SUPER_COLS)

    PB = NTAB // NBUCK
    NCK2 = PB // CH
    QCK = NCK2 // 4
    pos2 = ec * SHARD + (ep % CH) * NCHUNK + ep // CH
    eb2 = pos2 // PB
    lp2 = pos2 % PB
    cnt2 = np.bincount(c2 * NTAB + pos2, minlength=NC * NTAB)
    cnt2 = cnt2.reshape(NC, NBUCK, PB)
    pi2 = np.concatenate(
        [4 * np.argsort(-cnt2[:, :, r::4], axis=2, kind="stable") + r
         for r in range(4)], axis=2)
    inv2 = np.empty_like(pi2)
    np.put_along_axis(inv2, pi2,
                      np.broadcast_to(np.arange(PB), pi2.shape).copy(), axis=2)
    cnt2_sorted = np.take_along_axis(cnt2, pi2, axis=2)
    K2 = cnt2_sorted[:, :, ::CH].max(axis=0)
    cols2b = K2.sum(axis=1)
    base2 = np.concatenate([[0], np.cumsum(cols2b)]).astype(np.int64)
    TCOLS2 = int(base2[-1])
    CB2_MAX = int(cols2b.max())
    coloff2 = np.zeros((NBUCK, NCK2), dtype=np.int64)
    for b in range(NBUCK):
        coloff2[b] = base2[b] + np.r_[0, np.cumsum(K2[b])[:-1]]

    rp2 = inv2[c2, eb2, lp2]
    k2e = _cumcount((c2 * NBUCK + eb2) * PB + rp2, E)
    col2 = coloff2[eb2, rp2 // CH] + k2e
    slot2 = col2 * CH + (rp2 % CH)
    ell2_idx = np.zeros((NC, TCOLS2 * CH), dtype=np.int16)
    ell2_w = np.zeros((NC, TCOLS2 * CH), dtype=np.float32)
    ell2_idx[c2, slot2] = eloc2
    ell2_w[c2, slot2] = w
    ell2_idx_w = np.stack([_wrap_idx(ell2_idx[c]) for c in range(NC)])
    ell2_w_sb = np.ascontiguousarray(
        ell2_w.reshape(NC, TCOLS2, CH).transpose(0, 2, 1))

    agg2_groups = [[] for _ in range(NBUCK)]
    for b in range(NBUCK):
        runs = []
        for r in range(4):
            runs += [(q0 + r * QCK, g, K, int(coloff2[b, q0 + r * QCK]))
                     for (q0, g, K) in _runs_capped(
                         K2[b][r * QCK:(r + 1) * QCK], TILE_COLS, GCAP2)]
        agg2_groups[b] = _supers(runs, SUPER_COLS2)

    scat = np.empty((NC, CH, NBUCK * (PB // 16)), dtype=np.int16)
    for c in range(NC):
        for b in range(NBUCK):
            scat[c, :, b * (PB // 16):(b + 1) * (PB // 16)] = _wrap_idx(
                (pi2[c, b] // 4).astype(np.int16))

    p_v = pi % CH
    q_v = pi // CH
    row1 = p_v * (NCHUNK // 2) + q_v // 2
    comb = np.empty((NC, CH, NBUCK * (SHARD // 16)), dtype=np.int16)
    for c in range(NC):
        for b in range(NBUCK):
            comb[c, :, b * (SHARD // 16):(b + 1) * (SHARD // 16)] = _wrap_idx(
                row1[c, b].astype(np.int16))

    TSZ = CH
    for kdiv in range(32, 0, -1):
        if BUCK % (CH * kdiv) == 0:
            TSZ = CH * kdiv
            break
    VS = TSZ // CH
    NT = NTAB // TSZ
    TPB = NT // NBUCK
    indeg_tab = np.zeros(NTAB, dtype=np.int64)
    indeg_tab[tpos] = indeg
    KT = indeg_tab.reshape(NT, TSZ).max(axis=1)
    dega_groups = [[] for _ in range(NBUCK)]
    off = 0
    meta_T = np.zeros((NT, 3), dtype=np.int64)
    for b in range(NBUCK):
        for (Tl, g, K) in _runs_capped(KT[b * TPB:(b + 1) * TPB],
                                       max(DEG_CAP // VS, 1)):
            T0 = b * TPB + Tl
            dega_groups[b].append((T0, g, K, off))
            meta_T[T0:T0 + g] = (off, T0, g * VS * K)
            off += CH * g * VS * K
    LDEG = max(off, 16)
    wdeg = np.zeros(LDEG, dtype=np.float32)
    k2 = _cumcount(tdst, E)
    T_e = tdst // TSZ
    p_e = (tdst % TSZ) // VS
    s_e = tdst % VS
    pos = (meta_T[T_e, 0] + p_e * meta_T[T_e, 2]
           + ((T_e - meta_T[T_e, 1]) * VS + s_e) * KT[T_e] + k2)
    wdeg[pos] = w

    KQ = np.maximum(indeg_tab.reshape(NC, NCHUNK, CH).max(axis=(0, 2)), 0)
    degown_groups = []
    offo = 0
    meta_q = np.zeros((NCHUNK, 3), dtype=np.int64)
    for (q0, g, K) in _runs_capped(KQ, DEG_CAP):
        degown_groups.append((q0, g, K, offo))
        meta_q[q0:q0 + g] = (offo, q0, g * K)
        offo += CH * g * K
    LDEGO = max(offo, 16)
    wdeg_own = np.zeros((NC, LDEGO), dtype=np.float32)
    q_e = ep // CH
    pos_o = (meta_q[q_e, 0] + (ep % CH) * meta_q[q_e, 2]
             + (q_e - meta_q[q_e, 1]) * KQ[q_e] + k2)
    wdeg_own[ec, pos_o] = w

    meta = dict(
        N=N, E=E, DIN=DIN, DH=DH, DOUT=DOUT,
        SHARD=SHARD, NTAB=NTAB, BUCK=BUCK, NCHUNK=NCHUNK,
        TCOLS=TCOLS, LDEG=LDEG, LDEGO=LDEGO, CB_MAX=CB_MAX,
        TSZ=TSZ, VS=VS, NT=NT, TPB=TPB,
        PB=PB, NCK2=NCK2, TCOLS2=TCOLS2, CB2_MAX=CB2_MAX,
        bucket_base=tuple(int(v) for v in bucket_base),
        colsb=tuple(int(v) for v in colsb),
        base2=tuple(int(v) for v in base2),
        cols2b=tuple(int(v) for v in cols2b),
        agg_groups=tuple(tuple(g) for g in agg_groups),
        agg2_groups=tuple(tuple(g) for g in agg2_groups),
        dega_groups=tuple(tuple(g) for g in dega_groups),
        degown_groups=tuple(degown_groups),
    )

    in_maps = []
    for c in range(NC):
        in_maps.append({
            "x_perm": x_perm,
            "x_own": np.ascontiguousarray(x_own_pm[c]),
            "wdeg": wdeg.reshape(1, -1).astype(ml_dtypes.bfloat16),
            "wdeg_own": wdeg_own[c].reshape(1, -1).astype(ml_dtypes.bfloat16),
            "ell_idx": ell_idx_w[c],
            "ell_idx2": ell2_idx_w[c],
            "ell_w": ell_w_sb[c],
            "ell_w2": ell2_w_sb[c],
            "scat_idx": scat[c],
            "comb_idx": comb[c],
            "W1Tb": np.ascontiguousarray(
                np.asarray(W1, np.float32).T).astype(ml_dtypes.bfloat16),
            "W2Tb": np.ascontiguousarray(
                np.asarray(W2, np.float32).T).astype(ml_dtypes.bfloat16),
            "b1col": np.asarray(b1, np.float32).reshape(DH, 1).copy(),
            "b2bc": np.broadcast_to(np.asarray(b2, np.float32), (CH, DOUT)).copy(),
        })
    return meta, in_maps, tpos


def _build(meta):
    SHARD = meta["SHARD"]; NTAB = meta["NTAB"]; BUCK = meta["BUCK"]
    NCHUNK = meta["NCHUNK"]; TCOLS = meta["TCOLS"]; CB_MAX = meta["CB_MAX"]
    DIN = meta["DIN"]; DH = meta["DH"]; DOUT = meta["DOUT"]
    TSZ = meta["TSZ"]; VS = meta["VS"]; TPB = meta["TPB"]
    PB = meta["PB"]; TCOLS2 = meta["TCOLS2"]; CB2_MAX = meta["CB2_MAX"]
    BB = meta["bucket_base"]; BB2 = meta["base2"]
    CBW = max(CB_MAX, CB2_MAX)
    QCK = (PB // CH) // 4

    nc = bacc.Bacc(None, debug=True, num_swdge_queues=2)
    f32, i16, bf16 = mybir.dt.float32, mybir.dt.int16, mybir.dt.bfloat16

    x_perm = nc.dram_tensor("x_perm", [NTAB, DIN], bf16, kind="ExternalInput")
    x_own = nc.dram_tensor("x_own", [CH, NCHUNK * DIN], bf16, kind="ExternalInput")
    wdeg = nc.dram_tensor("wdeg", [1, meta["LDEG"]], bf16, kind="ExternalInput")
    wdeg_own = nc.dram_tensor("wdeg_own", [1, meta["LDEGO"]], bf16, kind="ExternalInput")
    ell_idx = nc.dram_tensor("ell_idx", [CH, TCOLS * 8], i16, kind="ExternalInput")
    ell_idx2 = nc.dram_tensor("ell_idx2", [CH, TCOLS2 * 8], i16, kind="ExternalInput")
    ell_w = nc.dram_tensor("ell_w", [CH, TCOLS], f32, kind="ExternalInput")
    ell_w2 = nc.dram_tensor("ell_w2", [CH, TCOLS2], f32, kind="ExternalInput")
    scat_idx = nc.dram_tensor("scat_idx", [CH, NBUCK * (PB // 16)], i16, kind="ExternalInput")
    comb_idx = nc.dram_tensor("comb_idx", [CH, NBUCK * (SHARD // 16)], i16, kind="ExternalInput")
    W1Tb = nc.dram_tensor("W1Tb", [DIN, DH], bf16, kind="ExternalInput")
    W2Tb = nc.dram_tensor("W2Tb", [DH, DOUT], bf16, kind="ExternalInput")
    b1col = nc.dram_tensor("b1col", [DH, 1], f32, kind="ExternalInput")
    b2bc = nc.dram_tensor("b2bc", [CH, DOUT], f32, kind="ExternalInput")
    out = nc.dram_tensor("out", [CH, NCHUNK * DOUT], f32, kind="ExternalOutput")

    mm_groups = [(s, min(MMG, NCHUNK - s)) for s in range(0, NCHUNK, MMG)]

    import contextlib
    with tile.TileContext(nc) as tc:
        with contextlib.ExitStack() as _stk:
            def _pool(name, bufs, space=None):
                kw = {"space": space} if space else {}
                return _stk.enter_context(
                    tc.tile_pool(name=name, bufs=bufs, **kw))
            cpool = _pool("const", 1)
            xtp = _pool("xt", 2)
            dpool = _pool("degt", 2)
            ipool = _pool("gidx", 2)
            wpool = _pool("wt", 2)
            w2pool = _pool("wt2", 2)
            gpool = _pool("gd", 3)
            rpool = _pool("red", 3)
            r2pool = _pool("red2", SCAT_LAG + 2)
            lpool = _pool("lred", 2)
            cgpool = _pool("cg", 2)

            spool = _pool("sl", 2)
            mpool = _pool("mm", 2)
            tpool = _pool("tp", 3)
            psum = _pool("psum", 2, "PSUM")
            psumtp = _pool("psumtp", 3, "PSUM")
            dram = _pool("dram", 1, "DRAM")
            nc.gpsimd.load_library(library_config.mlp)

            tables = [dram.tile([BUCK, DIN], f32, tag=f"tab{b}", name=f"tab{b}")
                      for b in range(NBUCK)]
            t2shard = dram.tile([CH, NCHUNK * DH], f32, tag="t2shard")
            partial2 = dram.tile([NC * CH, NCHUNK * DOUT], bf16, tag="partial2")
            rs_out = dram.tile([CH, NCHUNK * DOUT], bf16, tag="rs_out")
            acc1 = dram.tile([CH, NCHUNK * DH], bf16, tag="acc1")

            W1T_t = cpool.tile([DIN, DH], bf16, tag="w1t")
            W2b_t = cpool.tile([CH, DOUT], bf16, tag="w2t")
            b1c_t = cpool.tile([CH, 1], f32, tag="b1")
            b2_t = cpool.tile([CH, DOUT], f32, tag="b2")
            ident = cpool.tile([CH, CH], f32, tag="ident")
            identb = cpool.tile([CH, CH], bf16, tag="identb")
            comb_t = cpool.tile([CH, NBUCK * (SHARD // 16)], i16, tag="combt")
            wt_b = [None] * NBUCK
            wt2_b = [None] * NBUCK
            nc.sync.dma_start(out=W1T_t[:], in_=W1Tb[:, :])
            nc.sync.dma_start(out=W2b_t[:DH, :], in_=W2Tb[:, :])
            nc.sync.dma_start(out=W2b_t[DH:2 * DH, :], in_=W2Tb[:, :])
            nc.sync.dma_start(out=b1c_t[:DH, :], in_=b1col[:, :])
            nc.sync.dma_start(out=b1c_t[DH:2 * DH, :], in_=b1col[:, :])
            nc.sync.dma_start(out=b2_t[:], in_=b2bc[:, :])
            nc.sync.dma_start(out=comb_t[:], in_=comb_idx[:, :])
            make_identity(nc, ident[:])
            make_identity(nc, identb[:])

            ZW = NCHUNK * DOUT // 4
            zt = cpool.tile([CH, ZW], bf16, tag="zt")
            nc.vector.memset(zt[:], 0.0)

            def zero_steps():
                steps = []

                def za(j):
                    nc.sync.dma_start(
                        out=acc1[:, j * ZW:(j + 1) * ZW], in_=zt[:])

                def zw(a0, h):
                    nc.sync.dma_start(
                        out=partial2[a0 * CH:(a0 + 1) * CH,
                                     h * ZW:(h + 1) * ZW],
                        in_=zt[:])

                for j in range(NCHUNK * DH // ZW):
                    steps.append(lambda j_=j: za(j_))
                for a0 in range(NC):
                    for h in range(4):
                        steps.append(lambda a_=a0, h_=h: zw(a_, h_))
                return steps

            deg_a = cpool.tile([CH, meta["NT"] * VS], f32, tag="dega")
            dinv_a = cpool.tile([CH, meta["NT"] * VS], f32, tag="dinva")
            deg_o = cpool.tile([CH, NCHUNK], f32, tag="dego")
            dinv_o = cpool.tile([CH, NCHUNK], f32, tag="dinvo")
            nc.vector.memset(deg_a[:], 0.0)
            nc.vector.memset(deg_o[:], 0.0)
            for (q0, g, K, off) in meta["degown_groups"]:
                m = g * K
                t = dpool.tile([CH, DEG_CAP], bf16, tag="degt")
                nc.sync.dma_start(
                    out=t[:, :m],
                    in_=wdeg_own[0, off:off + CH * m].rearrange("(p m) -> p m", p=CH))
                nc.vector.tensor_reduce(
                    out=deg_o[:, q0:q0 + g],
                    in_=t[:, :m].rearrange("p (g k) -> p g k", g=g),
                    axis=mybir.AxisListType.X, op=mybir.AluOpType.add)
            nc.vector.tensor_scalar_add(out=deg_o[:], in0=deg_o[:], scalar1=1.0)
            nc.scalar.sqrt(out=dinv_o[:], in_=deg_o[:])
            nc.vector.reciprocal(out=dinv_o[:], in_=dinv_o[:])

            Dh = DH
            HC = NCHUNK // 2
            AV = 2 * DH
            scat1_q = []
            acc1rows = acc1[:, :].rearrange("p (u f) -> (p u) f", f=AV)

            def scat1_step(rt_, g_, q0_, b_):
                par = q0_ // HC
                i0 = b_ * (SHARD // 16)
                nc.gpsimd.dma_scatter_add(
                    out_ap=acc1rows[:, par * DH:(par + 1) * DH],
                    in_ap=rt_.rearrange("p (g d) -> p g d", g=g_),
                    idxs_ap=comb_t[:, i0 + q0_ * 8:i0 + (q0_ + g_) * 8],
                    num_idxs=CH * g_, num_idxs_reg=CH * g_, elem_size=DH,
                    elem_step=AV, single_packet=False, queue_num=1)

            sm1 = {"rt": None, "q0": -1, "fill": 0, "b": -1}

            def flush_scat1():
                if sm1["fill"]:
                    scat1_q.append(
                        lambda rt_=sm1["rt"][:, :sm1["fill"] * DH],
                        g_=sm1["fill"], q0_=sm1["q0"], b_=sm1["b"]:
                        scat1_step(rt_, g_, q0_, b_))
                    sm1["rt"] = None
                    sm1["fill"] = 0
                    while len(scat1_q) > SCAT_LAG:
                        scat1_q.pop(0)()

            def build_steps(b):
                steps = []

                def deg_step(T0, g, K, off):
                    m = g * VS * K
                    t = dpool.tile([CH, DEG_CAP], bf16, tag="degt", name="degt")
                    nc.sync.dma_start(
                        out=t[:, :m],
                        in_=wdeg[0, off:off + CH * m].rearrange(
                            "(p m) -> p m", p=CH))
                    nc.vector.tensor_reduce(
                        out=deg_a[:, T0 * VS:(T0 + g) * VS],
                        in_=t[:, :m].rearrange("p (v k) -> p v k", k=K),
                        axis=mybir.AxisListType.X, op=mybir.AluOpType.add)

                def dinv_step():
                    dsl = slice(b * TPB * VS, (b + 1) * TPB * VS)
                    nc.vector.tensor_scalar_add(
                        out=deg_a[:, dsl], in0=deg_a[:, dsl], scalar1=1.0)
                    nc.scalar.sqrt(out=dinv_a[:, dsl], in_=deg_a[:, dsl])
                    nc.vector.reciprocal(out=dinv_a[:, dsl], in_=dinv_a[:, dsl])

                def tile_step(tt):
                    T = b * TPB + tt
                    r0 = tt * TSZ
                    xt = xtp.tile([CH, VS * DIN], bf16, tag="xt", name="xt")
                    nc.sync.dma_start(
                        out=xt[:],
                        in_=x_perm[T * TSZ:(T + 1) * TSZ, :].rearrange(
                            "(p s) f -> p (s f)", p=CH))
                    xf = xtp.tile([CH, VS * DIN], f32, tag="xf", name="xf")
                    nc.vector.tensor_tensor(
                        out=xf[:].rearrange("p (s f) -> p s f", s=VS),
                        in0=xt[:].rearrange("p (s f) -> p s f", s=VS),
                        in1=dinv_a[:, T * VS:(T + 1) * VS][:, :, None]
                        .to_broadcast([CH, VS, DIN]),
                        op=mybir.AluOpType.mult)
                    nc.sync.dma_start(
                        out=tables[b][r0:r0 + TSZ, :].rearrange(
                            "(p s) f -> p (s f)", p=CH),
                        in_=xf[:])

                def stage_step():
                    cols_b = meta["colsb"][b]
                    it = ipool.tile([CH, CBW * 8], i16, tag="it", name="it")
                    nc.sync.dma_start(
                        out=it[:, :cols_b * 8],
                        in_=ell_idx[:, BB[b] * 8:(BB[b] + cols_b) * 8])
                    wt = wpool.tile([CH, CB_MAX], f32, tag="wt", name="wt")
                    nc.sync.dma_start(out=wt[:, :cols_b],
                                      in_=ell_w[:, BB[b]:BB[b] + cols_b])
                    wt_b[b] = wt
                    it_b[b] = it

                steps.append(stage_step)
                for (T0, g, K, off) in meta["dega_groups"][b]:
                    steps.append(lambda a=T0, bg=g, cK=K, do=off: deg_step(a, bg, cK, do))
                steps.append(dinv_step)
                for tt in range(TPB):
                    steps.append(lambda t_=tt: tile_step(t_))
                return steps

            st2_b = [None] * NBUCK

            def stage2(b):
                cols_b = meta["cols2b"][b]
                it = ipool.tile([CH, CBW * 8], i16, tag="it", name="it")
                nc.sync.dma_start(out=it[:, :cols_b * 8],
                                  in_=ell_idx2[:, BB2[b] * 8:(BB2[b] + cols_b) * 8])
                wt2 = w2pool.tile([CH, CB2_MAX], f32, tag="wt2", name="wt2")
                nc.sync.dma_start(out=wt2[:, :cols_b],
                                  in_=ell_w2[:, BB2[b]:BB2[b] + cols_b])
                wt2_b[b] = wt2
                sc = cgpool.tile([CH, PB // 16], i16, tag="sc", name="sc")
                nc.sync.dma_start(out=sc[:],
                                  in_=scat_idx[:, b * (PB // 16):(b + 1) * (PB // 16)])
                st2_b[b] = (it, sc)

            def mm_group(q0, sg):
                asl = spool.tile([CH, MMG * Dh], bf16, tag="aslb")
                nc.sync.dma_start(out=asl[:, :sg * Dh],
                                  in_=acc1[:, q0 * Dh:(q0 + sg) * Dh])
                a2 = spool.tile([CH, MMG * Dh], f32, tag="a2")
                a_sl = a2[:, :sg * Dh]
                st = spool.tile([CH, MMG * Dh], bf16, tag="st")
                nc.sync.dma_start(out=st[:, :sg * Dh],
                                  in_=x_own[:, q0 * Dh:(q0 + sg) * Dh])
                stf = spool.tile([CH, MMG * Dh], f32, tag="stf")
                nc.vector.tensor_tensor(
                    out=stf[:, :sg * Dh].rearrange("p (j f) -> p j f", j=sg),
                    in0=st[:, :sg * Dh].rearrange("p (j f) -> p j f", j=sg),
                    in1=dinv_o[:, q0:q0 + sg][:, :, None]
                    .to_broadcast([CH, sg, Dh]),
                    op=mybir.AluOpType.mult)
                nc.vector.tensor_tensor(out=a_sl, in0=asl[:, :sg * Dh],
                                        in1=stf[:, :sg * Dh],
                                        op=mybir.AluOpType.add)
                nc.vector.tensor_tensor(
                    out=a_sl.rearrange("p (j f) -> p j f", j=sg),
                    in0=a_sl.rearrange("p (j f) -> p j f", j=sg),
                    in1=dinv_o[:, q0:q0 + sg][:, :, None]
                    .to_broadcast([CH, sg, Dh]),
                    op=mybir.AluOpType.mult)
                mm_ps = psum.tile([CH, MMG * DOUT], f32, tag="mmps", space="PSUM")
                a2b = spool.tile([CH, MMG * Dh], bf16, tag="a2b")
                nc.vector.tensor_copy(out=a2b[:, :sg * Dh],
                                      in_=a2[:, :sg * Dh])
                rls = []
                for jj0 in range(0, sg, 4):
                    bw = min(4, sg - jj0)
                    tp_ps = psumtp.tile([CH, 4 * CH], bf16, tag="tpps",
                                        space="PSUM")
                    for t in range(bw):
                        nc.tensor.transpose(
                            out=tp_ps[:Dh, t * CH:(t + 1) * CH],
                            in_=a2b[:, (jj0 + t) * Dh:(jj0 + t + 1) * Dh],
                            identity=identb[:])
                    tp_sb = tpool.tile([CH, 4 * CH], bf16, tag="tpsb")
                    nc.scalar.copy(out=tp_sb[:Dh, :bw * CH],
                                   in_=tp_ps[:Dh, :bw * CH])
                    h1_ps = psumtp.tile([CH, 4 * CH], f32, tag="h1ps",
                                        space="PSUM")
                    for t in range(bw):
                        nc.tensor.matmul(
                            out=h1_ps[:DH, t * CH:(t + 1) * CH],
                            lhsT=W1T_t[:DIN, :],
                            rhs=tp_sb[:Dh, t * CH:(t + 1) * CH],
                            start=True, stop=True)
                    rl_sb = tpool.tile([CH, 4 * CH], bf16, tag="rlsb")
                    nc.scalar.activation(
                        out=rl_sb[:DH, :bw * CH], in_=h1_ps[:DH, :bw * CH],
                        func=mybir.ActivationFunctionType.Relu,
                        bias=b1c_t[:DH, 0:1])
                    rls.append(rl_sb)
                for jj in range(sg):
                    nc.tensor.matmul(
                        out=mm_ps[:, jj * DOUT:(jj + 1) * DOUT],
                        lhsT=rls[jj // 4][:DH, (jj % 4) * CH:(jj % 4 + 1) * CH],
                        rhs=W2b_t[:DH, :],
                        start=True, stop=True)
                h = mpool.tile([CH, MMG * DOUT], f32, tag="hmm")
                nc.vector.tensor_tensor(
                    out=h[:, :sg * DOUT].rearrange("p (j f) -> p j f", j=sg),
                    in0=mm_ps[:, :sg * DOUT].rearrange("p (j f) -> p j f", j=sg),
                    in1=dinv_o[:, q0:q0 + sg][:, :, None]
                    .to_broadcast([CH, sg, DOUT]),
                    op=mybir.AluOpType.mult)
                nc.sync.dma_start(
                    out=t2shard[:, q0 * DH:(q0 + sg) * DH].rearrange(
                        "p (j f) -> p j f", j=sg)[:, :, :DOUT],
                    in_=h[:, :sg * DOUT].rearrange("p (j f) -> p j f", j=sg))

            it_b = [None] * NBUCK
            for s in build_steps(0):
                s()
            deferred = []
            zs_all = zero_steps()
            zdefer = zs_all[24:]
            for b in range(NBUCK):
                if b + 1 < NBUCK:
                    deferred = build_steps(b + 1)
                if b == 0:
                    deferred = deferred + zs_all[:24]
                it = it_b[b]
                for gi, (c0, cols, runs) in enumerate(meta["agg_groups"][b]):
                    cl = c0 - BB[b]
                    gd = gpool.tile([CH, TILE_COLS * Dh], f32, tag="gd")
                    nc.gpsimd.dma_gather(
                        out_ap=gd[:, :cols * Dh].rearrange("p (c d) -> p c d", c=cols),
                        in_ap=tables[b][:, :],
                        idxs_ap=it[:, cl * 8:(cl + cols) * 8],
                        num_idxs=CH * cols, num_idxs_reg=CH * cols, elem_size=Dh,
                        single_packet=False)
                    if len(runs) == 1 and runs[0][2] == 1:
                        kone1 = True
                        (q0, g, K, csg) = runs[0]
                    else:
                        kone1 = False
                        nc.vector.tensor_tensor(
                            out=gd[:, :cols * Dh].rearrange("p (c d) -> p c d", c=cols),
                            in0=gd[:, :cols * Dh].rearrange("p (c d) -> p c d", c=cols),
                            in1=wt_b[b][:, cl:cl + cols][:, :, None]
                            .to_broadcast([CH, cols, Dh]),
                            op=mybir.AluOpType.mult)
                    for (q0, g, K, csg) in runs:
                        off = csg - c0
                        if (sm1["fill"] == 0 or sm1["b"] != b
                                or q0 != sm1["q0"] + sm1["fill"]
                                or q0 // HC != sm1["q0"] // HC
                                or sm1["fill"] + g > SCMERGE1):
                            flush_scat1()
                            sm1.update(rt=rpool.tile([CH, SCMERGE1 * Dh],
                                                     bf16, tag="rt",
                                                     name="rt"),
                                       q0=q0, fill=0, b=b)
                        rt = sm1["rt"]
                        f0 = sm1["fill"]
                        rsl = rt[:, f0 * Dh:(f0 + g) * Dh]
                        if kone1:
                            nc.vector.tensor_tensor(
                                out=rsl.rearrange("p (c d) -> p c d", c=g),
                                in0=gd[:, off * Dh:(off + g) * Dh].rearrange(
                                    "p (c d) -> p c d", c=g),
                                in1=wt_b[b][:, cl + off:cl + off + g]
                                [:, :, None].to_broadcast([CH, g, Dh]),
                                op=mybir.AluOpType.mult)
                        else:
                            with nc.allow_low_precision("bf16 L1 acc"):
                                nc.vector.tensor_reduce(
                                    out=rsl.rearrange("p (g d) -> p g d", g=g),
                                    in_=gd[:, off * Dh:(off + g * K) * Dh]
                                    .rearrange("p (g k d) -> p g d k",
                                               g=g, k=K),
                                    axis=mybir.AxisListType.X,
                                    op=mybir.AluOpType.add)
                        sm1["fill"] += g
                    for _ in range(6):
                        if deferred:
                            deferred.pop(0)()
                while deferred:
                    deferred.pop(0)()
            flush_scat1()
            while scat1_q:
                scat1_q.pop(0)()

            for s_ in range(NS):
                mm_group(*mm_groups[s_])
                for _ in range(2):
                    if zdefer:
                        zdefer.pop(0)()
                if s_ == 0:
                    stage2(0)
                if s_ == 2:
                    stage2(1)

            QD = 4 * DOUT
            t2rows = t2shard[:, :].rearrange("p (u f) -> (p u) f", f=DH)
            scat_q = []

            def scat_step(rt_, g_, q0_, b_, sc_):
                pv = partial2[b_ * CPB * CH:(b_ + 1) * CPB * CH, :].rearrange(
                    "a (u f) -> (a u) f", f=QD)
                cls = q0_ // QCK
                nc.gpsimd.dma_scatter_add(
                    out_ap=pv[:, cls * DOUT:(cls + 1) * DOUT],
                    in_ap=rt_.rearrange("p (g d) -> p g d", g=g_),
                    idxs_ap=sc_[:, q0_ * 8:(q0_ + g_) * 8],
                    num_idxs=CH * g_, num_idxs_reg=CH * g_, elem_size=DOUT,
                    elem_step=QD,
                    single_packet=False, queue_num=1)

            sm = {"rt": None, "q0": -1, "fill": 0, "b": -1, "sc": None}

            def flush_scat():
                if sm["fill"]:
                    scat_q.append(
                        lambda rt_=sm["rt"][:, :sm["fill"] * DOUT],
                        g_=sm["fill"], q0_=sm["q0"], b_=sm["b"],
                        sc_=sm["sc"]: scat_step(rt_, g_, q0_, b_, sc_))
                    sm["rt"] = None
                    sm["fill"] = 0
                    while len(scat_q) > SCAT_LAG:
                        scat_q.pop(0)()

            for b in range(NBUCK):
                it, sc = st2_b[b]
                for g2i, (c0, cols, runs) in enumerate(meta["agg2_groups"][b]):
                    cl = c0 - BB2[b]
                    gd = gpool.tile([CH, TILE_COLS * DH], f32, tag="gd")
                    nc.gpsimd.dma_gather(
                        out_ap=gd[:, :cols * DH].rearrange("p (c d) -> p c d", c=cols),
                        in_ap=t2rows,
                        idxs_ap=it[:, cl * 8:(cl + cols) * 8],
                        num_idxs=CH * cols, num_idxs_reg=CH * cols, elem_size=DH,
                        single_packet=False)
                    kone = len(runs) == 1 and runs[0][2] == 1
                    if not kone:
                        nc.vector.tensor_tensor(
                            out=gd[:, :cols * DH].rearrange(
                                "p (c d) -> p c d", c=cols)[:, :, :DOUT],
                            in0=gd[:, :cols * DH].rearrange(
                                "p (c d) -> p c d", c=cols)[:, :, :DOUT],
                            in1=wt2_b[b][:, cl:cl + cols][:, :, None]
                            .to_broadcast([CH, cols, DOUT]),
                            op=mybir.AluOpType.mult)
                    for (q0, g, K, csg) in runs:
                        off = csg - c0
                        if (sm["fill"] == 0 or sm["b"] != b
                                or q0 != sm["q0"] + sm["fill"]
                                or q0 // QCK != sm["q0"] // QCK
                                or sm["fill"] + g > SCMERGE):
                            flush_scat()
                            sm.update(rt=r2pool.tile([CH, SCMERGE * DOUT],
                                                     bf16, tag="rt2b",
                                                     name="rtb"),
                                      q0=q0, fill=0, b=b, sc=sc)
                        rt = sm["rt"]
                        f0 = sm["fill"]
                        rsl = rt[:, f0 * DOUT:(f0 + g) * DOUT]
                        if kone:
                            nc.vector.tensor_tensor(
                                out=rsl.rearrange("p (g d) -> p g d", g=g),
                                in0=gd[:, off * DH:(off + g) * DH].rearrange(
                                    "p (g d) -> p g d", g=g)[:, :, :DOUT],
                                in1=wt2_b[b][:, cl + off:cl + off + g]
                                [:, :, None].to_broadcast([CH, g, DOUT]),
                                op=mybir.AluOpType.mult)
                        else:
                            with nc.allow_low_precision("bf16 L2 partials"):
                                nc.vector.tensor_reduce(
                                    out=rsl.rearrange("p (g d) -> p g d", g=g),
                                    in_=gd[:, off * DH:(off + g * K) * DH]
                                    .rearrange("p (g k d) -> p g d k",
                                               g=g, k=K)[:, :, :DOUT, :],
                                    axis=mybir.AxisListType.X,
                                    op=mybir.AluOpType.add)
                        sm["fill"] += g
                    if g2i == 0 and b + 1 < NBUCK:
                        stage2(b + 1)
            flush_scat()
            while scat_q:
                scat_q.pop(0)()

            nc.gpsimd.collective_compute(
                "ReduceScatter", mybir.AluOpType.add,
                replica_groups=[list(range(NC))],
                ins=[partial2[:, :].opt()],
                outs=[rs_out[:, :].opt()])

            TH = NCHUNK // 5
            for hh in range(5):
                csl = slice(hh * TH * DOUT, (hh + 1) * TH * DOUT)
                rsb = spool.tile([CH, TH * DOUT], bf16, tag="rsb")
                nc.sync.dma_start(out=rsb[:], in_=rs_out[:, csl])
                t2b = spool.tile([CH, TH * DOUT], f32, tag="t2b")
                nc.sync.dma_start(
                    out=t2b[:].rearrange("p (j f) -> p j f", j=TH),
                    in_=t2shard[:, hh * TH * DH:(hh + 1) * TH * DH].rearrange(
                        "p (j f) -> p j f", j=TH)[:, :, :DOUT])
                fin = mpool.tile([CH, TH * DOUT], f32, tag="fin")
                nc.vector.tensor_copy(out=fin[:], in_=rsb[:])
                nc.vector.tensor_tensor(out=fin[:], in0=fin[:], in1=t2b[:],
                                        op=mybir.AluOpType.add)
                nc.vector.tensor_tensor(
                    out=fin[:].rearrange("p (j f) -> p j f", j=TH),
                    in0=fin[:].rearrange("p (j f) -> p j f", j=TH),
                    in1=dinv_o[:, hh * TH:(hh + 1) * TH][:, :, None]
                    .to_broadcast([CH, TH, DOUT]),
                    op=mybir.AluOpType.mult)
                nc.vector.tensor_tensor(
                    out=fin[:].rearrange("p (j f) -> p j f", j=TH),
                    in0=fin[:].rearrange("p (j f) -> p j f", j=TH),
                    in1=b2_t[:, None, :].to_broadcast([CH, TH, DOUT]),
                    op=mybir.AluOpType.add)
                nc.sync.dma_start(out=out[:, csl], in_=fin[:])

    nc.compile()
    _split_multi_waits(nc)
    return nc


_CACHE = {}


def kernel(x, edge_index, edge_weights, W1, b1, W2, b2):
    x = np.asarray(x); edge_index = np.asarray(edge_index)
    edge_weights = np.asarray(edge_weights)
    W1 = np.asarray(W1); b1 = np.asarray(b1)
    W2 = np.asarray(W2); b2 = np.asarray(b2)

    meta, in_maps, tpos = _prep(x, edge_index, edge_weights, W1, b1, W2, b2)
    key = (x.shape, edge_index.shape, meta["TCOLS"], meta["TCOLS2"],
           meta["LDEG"], meta["LDEGO"], meta["agg_groups"], meta["agg2_groups"],
           meta["dega_groups"], meta["degown_groups"])
    if key not in _CACHE:
        _CACHE[key] = _build(meta)
    nc = _CACHE[key]
    res = run_bass_kernel_spmd(nc, in_maps, list(range(NC)))
    NCHUNK, DOUT, SHARD = meta["NCHUNK"], meta["DOUT"], meta["SHARD"]
    blocks = [res.results[c]["out"].reshape(CH, NCHUNK, DOUT).transpose(1, 0, 2)
              .reshape(SHARD, DOUT) for c in range(NC)]
    full = np.concatenate(blocks, axis=0)
    return full[tpos].astype(np.float32)



# revision 140
# speedup vs baseline: 1.0021x; 1.0021x over previous
import numpy as np
import ml_dtypes

import concourse.bass as bass
import concourse.bacc as bacc
import concourse.mybir as mybir
import concourse.tile as tile
from concourse import library_config
from concourse.bass_utils import run_bass_kernel_spmd
from concourse.masks import make_identity

NC = 8
CH = 128
NBUCK = 4
CPB = NC // NBUCK
TILE_COLS = 80
GCAP = 16
GCAP2 = 24
SUPER_COLS = 1
SUPER_COLS2 = 1
SCMERGE = 48
SCMERGE1 = 24
SCAT_LAG = 2
DEG_CAP = 1024
MMG = 10
NS = 10


def _split_multi_waits(nc, max_waits=1):
    n = 0
    for f in nc.m.functions:
        for bb in f.blocks:
            new_insts = []
            for inst in bb.instructions:
                si = inst.sync_info
                if si is not None and si.on_wait and len(si.on_wait) > max_waits:
                    waits = list(si.on_wait)
                    for w in waits[max_waits:]:
                        n += 1
                        new_insts.append(mybir.InstNoOp(
                            name=f"WSPLIT-{n}-{inst.name}",
                            sync_info=mybir.SyncInfo(on_wait=[w], on_update=[]),
                            bass_nofuse=True,
                            engine=inst.engine,
                        ))
                    si.on_wait = waits[:max_waits]
                new_insts.append(inst)
            bb.instructions.clear()
            for i in new_insts:
                bb.add_instruction(i)
    return n


def _wrap_idx(lst):
    w = lst.reshape(-1, 16).T
    return np.ascontiguousarray(np.tile(w, (8, 1)))


def _cumcount(key, n):
    o = np.argsort(key, kind="stable")
    sk = key[o]
    starts = np.flatnonzero(np.r_[True, sk[1:] != sk[:-1]])
    lens = np.diff(np.r_[starts, n])
    kk = np.arange(n, dtype=np.int64) - np.repeat(starts, lens)
    k = np.empty(n, dtype=np.int64)
    k[o] = kk
    return k


def _runs_capped(K, cap, gcap=None):
    groups = []
    q, n = 0, len(K)
    while q < n:
        k = int(K[q])
        if k == 0:
            q += 1
            continue
        g = 1
        lim = max(1, cap // k)
        if gcap is not None:
            lim = min(lim, gcap)
        while q + g < n and int(K[q + g]) == k and g < lim:
            g += 1
        groups.append((q, g, k))
        q += g
    return groups


def _prep(x, edge_index, edge_weights, W1, b1, W2, b2):
    N, DIN = x.shape
    DH = W1.shape[0]
    DOUT = W2.shape[0]
    E = edge_index.shape[1]
    assert DIN == DH, "partial buffers assume DIN == DH"
    per_core = -(-N // NC)
    SHARD = -(-per_core // (4 * CH)) * (4 * CH)
    NTAB = NC * SHARD
    BUCK = NTAB // NBUCK
    NCHUNK = SHARD // CH
    assert BUCK <= 32767 and NC % NBUCK == 0

    src = np.asarray(edge_index[0], dtype=np.int64)
    dst = np.asarray(edge_index[1], dtype=np.int64)
    w = np.asarray(edge_weights, dtype=np.float32)

    indeg = np.bincount(dst, minlength=N)
    order = np.argsort(-indeg, kind="stable")
    i_of = np.empty(N, dtype=np.int64)
    i_of[order] = np.arange(N)
    tpos = (i_of % NC) * SHARD + (i_of // NC)

    x_perm = np.zeros((NTAB, DIN), dtype=np.float32)
    x_perm[tpos] = np.asarray(x, dtype=np.float32)
    x_perm = x_perm.astype(ml_dtypes.bfloat16)
    x_own_pm = (x_perm.reshape(NC, NCHUNK, CH, DIN).transpose(0, 2, 1, 3)
                .reshape(NC, CH, NCHUNK * DIN))

    tsrc = tpos[src]
    tdst = tpos[dst]
    ec = tdst // SHARD
    ep = tdst % SHARD
    eb = tsrc // BUCK
    eloc = (tsrc % BUCK).astype(np.int16)
    c2 = tsrc // SHARD
    ps_ = (tsrc % SHARD) % CH
    qs_ = (tsrc % SHARD) // CH
    eloc2 = (ps_ * NCHUNK + qs_).astype(np.int16)

    cb = ec * NBUCK + eb
    cnt = np.bincount(cb * SHARD + ep, minlength=NC * NBUCK * SHARD)
    cnt = cnt.reshape(NC, NBUCK, SHARD)
    sq = (np.arange(SHARD) // CH) % 2
    par_regions = [np.flatnonzero(sq == p_) for p_ in (0, 1)]
    pi = np.concatenate(
        [reg[np.argsort(-cnt[:, :, reg], axis=2, kind="stable")]
         for reg in par_regions], axis=2)
    inv = np.empty_like(pi)
    np.put_along_axis(inv, pi, np.broadcast_to(np.arange(SHARD), pi.shape).copy(), axis=2)
    cnt_sorted = np.take_along_axis(cnt, pi, axis=2)
    Kb = np.maximum(cnt_sorted[:, :, ::CH].max(axis=0), 1)
    assert int(Kb.max()) <= TILE_COLS
    colsb = Kb.sum(axis=1)
    bucket_base = np.concatenate([[0], np.cumsum(colsb)]).astype(np.int64)
    TCOLS = int(bucket_base[-1])
    CB_MAX = int(colsb.max())
    coloff = np.zeros((NBUCK, NCHUNK), dtype=np.int64)
    for b in range(NBUCK):
        coloff[b] = bucket_base[b] + np.r_[0, np.cumsum(Kb[b])[:-1]]

    rp = inv[ec, eb, ep]
    k = _cumcount(cb * SHARD + rp, E)
    col = coloff[eb, rp // CH] + k
    slot = col * CH + (rp % CH)
    ell_idx = np.zeros((NC, TCOLS * CH), dtype=np.int16)
    ell_w = np.zeros((NC, TCOLS * CH), dtype=np.float32)
    ell_idx[ec, slot] = eloc
    ell_w[ec, slot] = w

    ell_idx_w = np.stack([_wrap_idx(ell_idx[c]) for c in range(NC)])
    ell_w_sb = np.ascontiguousarray(
        ell_w.reshape(NC, TCOLS, CH).transpose(0, 2, 1))

    def _supers(runs, cap):
        supers = []
        cur, cur_cols, c0 = [], 0, None
        for (q0, g, K, cs) in runs:
            cols = g * K
            if cur and cur_cols + cols > cap:
                supers.append((c0, cur_cols, tuple(cur)))
                cur, cur_cols = [], 0
            if not cur:
                c0 = cs
            cur.append((q0, g, K, cs))
            cur_cols += cols
        if cur:
            supers.append((c0, cur_cols, tuple(cur)))
        return supers

    agg_groups = [[] for _ in range(NBUCK)]
    HC = NCHUNK // 2
    for b in range(NBUCK):
        runs = []
        for p_ in range(2):
            runs += [(q0 + p_ * HC, g, K, int(coloff[b, q0 + p_ * HC]))
                     for (q0, g, K) in _runs_capped(
                         Kb[b][p_ * HC:(p_ + 1) * HC], TILE_COLS, GCAP)]
        agg_groups[b] = _supers(runs, SUPER_COLS)

    PB = NTAB // NBUCK
    NCK2 = PB // CH
    QCK = NCK2 // 4
    pos2 = ec * SHARD + (ep % CH) * NCHUNK + ep // CH
    eb2 = pos2 // PB
    lp2 = pos2 % PB
    cnt2 = np.bincount(c2 * NTAB + pos2, minlength=NC * NTAB)
    cnt2 = cnt2.reshape(NC, NBUCK, PB)
    pi2 = np.concatenate(
        [4 * np.argsort(-cnt2[:, :, r::4], axis=2, kind="stable") + r
         for r in range(4)], axis=2)
    inv2 = np.empty_like(pi2)
    np.put_along_axis(inv2, pi2,
                      np.broadcast_to(np.arange(PB), pi2.shape).copy(), axis=2)
    cnt2_sorted = np.take_along_axis(cnt2, pi2, axis=2)
    K2 = cnt2_sorted[:, :, ::CH].max(axis=0)
    cols2b = K2.sum(axis=1)
    base2 = np.concatenate([[0], np.cumsum(cols2b)]).astype(np.int64)
    TCOLS2 = int(base2[-1])
    CB2_MAX = int(cols2b.max())
    coloff2 = np.zeros((NBUCK, NCK2), dtype=np.int64)
    for b in range(NBUCK):
        coloff2[b] = base2[b] + np.r_[0, np.cumsum(K2[b])[:-1]]

    rp2 = inv2[c2, eb2, lp2]
    k2e = _cumcount((c2 * NBUCK + eb2) * PB + rp2, E)
    col2 = coloff2[eb2, rp2 // CH] + k2e
    slot2 = col2 * CH + (rp2 % CH)
    ell2_idx = np.zeros((NC, TCOLS2 * CH), dtype=np.int16)
    ell2_w = np.zeros((NC, TCOLS2 * CH), dtype=np.float32)
    ell2_idx[c2, slot2] = eloc2
    ell2_w[c2, slot2] = w
    ell2_idx_w = np.stack([_wrap_idx(ell2_idx[c]) for c in range(NC)])
    ell2_w_sb = np.ascontiguousarray(
        ell2_w.reshape(NC, TCOLS2, CH).transpose(0, 2, 1))

    agg2_groups = [[] for _ in range(NBUCK)]
    for b in range(NBUCK):
        runs = []
        for r in range(4):
            runs += [(q0 + r * QCK, g, K, int(coloff2[b, q0 + r * QCK]))
                     for (q0, g, K) in _runs_capped(
                         K2[b][r * QCK:(r + 1) * QCK], TILE_COLS, GCAP2)]
        agg2_groups[b] = _supers(runs, SUPER_COLS2)

    scat = np.empty((NC, CH, NBUCK * (PB // 16)), dtype=np.int16)
    for c in range(NC):
        for b in range(NBUCK):
            scat[c, :, b * (PB // 16):(b + 1) * (PB // 16)] = _wrap_idx(
                (pi2[c, b] // 4).astype(np.int16))

    p_v = pi % CH
    q_v = pi // CH
    row1 = p_v * (NCHUNK // 2) + q_v // 2
    comb = np.empty((NC, CH, NBUCK * (SHARD // 16)), dtype=np.int16)
    for c in range(NC):
        for b in range(NBUCK):
            comb[c, :, b * (SHARD // 16):(b + 1) * (SHARD // 16)] = _wrap_idx(
                row1[c, b].astype(np.int16))

    TSZ = CH
    for kdiv in range(32, 0, -1):
        if BUCK % (CH * kdiv) == 0:
            TSZ = CH * kdiv
            break
    VS = TSZ // CH
    NT = NTAB // TSZ
    TPB = NT // NBUCK
    indeg_tab = np.zeros(NTAB, dtype=np.int64)
    indeg_tab[tpos] = indeg
    KT = indeg_tab.reshape(NT, TSZ).max(axis=1)
    dega_groups = [[] for _ in range(NBUCK)]
    off = 0
    meta_T = np.zeros((NT, 3), dtype=np.int64)
    for b in range(NBUCK):
        for (Tl, g, K) in _runs_capped(KT[b * TPB:(b + 1) * TPB],
                                       max(DEG_CAP // VS, 1)):
            T0 = b * TPB + Tl
            dega_groups[b].append((T0, g, K, off))
            meta_T[T0:T0 + g] = (off, T0, g * VS * K)
            off += CH * g * VS * K
    LDEG = max(off, 16)
    wdeg = np.zeros(LDEG, dtype=np.float32)
    k2 = _cumcount(tdst, E)
    T_e = tdst // TSZ
    p_e = (tdst % TSZ) // VS
    s_e = tdst % VS
    pos = (meta_T[T_e, 0] + p_e * meta_T[T_e, 2]
           + ((T_e - meta_T[T_e, 1]) * VS + s_e) * KT[T_e] + k2)
    wdeg[pos] = w

    KQ = np.maximum(indeg_tab.reshape(NC, NCHUNK, CH).max(axis=(0, 2)), 0)
    degown_groups = []
    offo = 0
    meta_q = np.zeros((NCHUNK, 3), dtype=np.int64)
    for (q0, g, K) in _runs_capped(KQ, DEG_CAP):
        degown_groups.append((q0, g, K, offo))
        meta_q[q0:q0 + g] = (offo, q0, g * K)
        offo += CH * g * K
    LDEGO = max(offo, 16)
    wdeg_own = np.zeros((NC, LDEGO), dtype=np.float32)
    q_e = ep // CH
    pos_o = (meta_q[q_e, 0] + (ep % CH) * meta_q[q_e, 2]
             + (q_e - meta_q[q_e, 1]) * KQ[q_e] + k2)
    wdeg_own[ec, pos_o] = w

    meta = dict(
        N=N, E=E, DIN=DIN, DH=DH, DOUT=DOUT,
        SHARD=SHARD, NTAB=NTAB, BUCK=BUCK, NCHUNK=NCHUNK,
        TCOLS=TCOLS, LDEG=LDEG, LDEGO=LDEGO, CB_MAX=CB_MAX,
        TSZ=TSZ, VS=VS, NT=NT, TPB=TPB,
        PB=PB, NCK2=NCK2, TCOLS2=TCOLS2, CB2_MAX=CB2_MAX,
        bucket_base=tuple(int(v) for v in bucket_base),
        colsb=tuple(int(v) for v in colsb),
        base2=tuple(int(v) for v in base2),
        cols2b=tuple(int(v) for v in cols2b),
        agg_groups=tuple(tuple(g) for g in agg_groups),
        agg2_groups=tuple(tuple(g) for g in agg2_groups),
        dega_groups=tuple(tuple(g) for g in dega_groups),
        degown_groups=tuple(degown_groups),
    )

    in_maps = []
    for c in range(NC):
        in_maps.append({
            "x_perm": x_perm,
            "x_own": np.ascontiguousarray(x_own_pm[c]),
            "wdeg": wdeg.reshape(1, -1).astype(ml_dtypes.bfloat16),
            "wdeg_own": wdeg_own[c].reshape(1, -1).astype(ml_dtypes.bfloat16),
            "ell_idx": ell_idx_w[c],
            "ell_idx2": ell2_idx_w[c],
            "ell_w": ell_w_sb[c],
            "ell_w2": ell2_w_sb[c],
            "scat_idx": scat[c],
            "comb_idx": comb[c],
            "W1Tb": np.ascontiguousarray(
                np.asarray(W1, np.float32).T).astype(ml_dtypes.bfloat16),
            "W2Tb": np.ascontiguousarray(
                np.asarray(W2, np.float32).T).astype(ml_dtypes.bfloat16),
            "b1col": np.asarray(b1, np.float32).reshape(DH, 1).copy(),
            "b2bc": np.broadcast_to(np.asarray(b2, np.float32), (CH, DOUT)).copy(),
        })
    return meta, in_maps, tpos


def _build(meta):
    SHARD = meta["SHARD"]; NTAB = meta["NTAB"]; BUCK = meta["BUCK"]
    NCHUNK = meta["NCHUNK"]; TCOLS = meta["TCOLS"]; CB_MAX = meta["CB_MAX"]
    DIN = meta["DIN"]; DH = meta["DH"]; DOUT = meta["DOUT"]
    TSZ = meta["TSZ"]; VS = meta["VS"]; TPB = meta["TPB"]
    PB = meta["PB"]; TCOLS2 = meta["TCOLS2"]; CB2_MAX = meta["CB2_MAX"]
    BB = meta["bucket_base"]; BB2 = meta["base2"]
    CBW = max(CB_MAX, CB2_MAX)
    QCK = (PB // CH) // 4

    nc = bacc.Bacc(None, debug=True, num_swdge_queues=2)
    f32, i16, bf16 = mybir.dt.float32, mybir.dt.int16, mybir.dt.bfloat16

    x_perm = nc.dram_tensor("x_perm", [NTAB, DIN], bf16, kind="ExternalInput")
    x_own = nc.dram_tensor("x_own", [CH, NCHUNK * DIN], bf16, kind="ExternalInput")
    wdeg = nc.dram_tensor("wdeg", [1, meta["LDEG"]], bf16, kind="ExternalInput")
    wdeg_own = nc.dram_tensor("wdeg_own", [1, meta["LDEGO"]], bf16, kind="ExternalInput")
    ell_idx = nc.dram_tensor("ell_idx", [CH, TCOLS * 8], i16, kind="ExternalInput")
    ell_idx2 = nc.dram_tensor("ell_idx2", [CH, TCOLS2 * 8], i16, kind="ExternalInput")
    ell_w = nc.dram_tensor("ell_w", [CH, TCOLS], f32, kind="ExternalInput")
    ell_w2 = nc.dram_tensor("ell_w2", [CH, TCOLS2], f32, kind="ExternalInput")
    scat_idx = nc.dram_tensor("scat_idx", [CH, NBUCK * (PB // 16)], i16, kind="ExternalInput")
    comb_idx = nc.dram_tensor("comb_idx", [CH, NBUCK * (SHARD // 16)], i16, kind="ExternalInput")
    W1Tb = nc.dram_tensor("W1Tb", [DIN, DH], bf16, kind="ExternalInput")
    W2Tb = nc.dram_tensor("W2Tb", [DH, DOUT], bf16, kind="ExternalInput")
    b1col = nc.dram_tensor("b1col", [DH, 1], f32, kind="ExternalInput")
    b2bc = nc.dram_tensor("b2bc", [CH, DOUT], f32, kind="ExternalInput")
    out = nc.dram_tensor("out", [CH, NCHUNK * DOUT], f32, kind="ExternalOutput")

    mm_groups = [(s, min(MMG, NCHUNK - s)) for s in range(0, NCHUNK, MMG)]

    import contextlib
    with tile.TileContext(nc) as tc:
        with contextlib.ExitStack() as _stk:
            def _pool(name, bufs, space=None):
                kw = {"space": space} if space else {}
                return _stk.enter_context(
                    tc.tile_pool(name=name, bufs=bufs, **kw))
            cpool = _pool("const", 1)
            xtp = _pool("xt", 2)
            dpool = _pool("degt", 2)
            ipool = _pool("gidx", 2)
            wpool = _pool("wt", 2)
            w2pool = _pool("wt2", 2)
            gpool = _pool("gd", 3)
            rpool = _pool("red", 3)
            r2pool = _pool("red2", SCAT_LAG + 2)
            lpool = _pool("lred", 2)
            cgpool = _pool("cg", 2)

            spool = _pool("sl", 2)
            mpool = _pool("mm", 2)
            tpool = _pool("tp", 3)
            psum = _pool("psum", 2, "PSUM")
            psumtp = _pool("psumtp", 3, "PSUM")
            dram = _pool("dram", 1, "DRAM")
            nc.gpsimd.load_library(library_config.mlp)

            tables = [dram.tile([BUCK, DIN], f32, tag=f"tab{b}", name=f"tab{b}")
                      for b in range(NBUCK)]
            t2shard = dram.tile([CH, NCHUNK * DH], f32, tag="t2shard")
            partial2 = dram.tile([NC * CH, NCHUNK * DOUT], bf16, tag="partial2")
            rs_out = dram.tile([CH, NCHUNK * DOUT], bf16, tag="rs_out")
            acc1 = dram.tile([CH, NCHUNK * DH], bf16, tag="acc1")

            W1T_t = cpool.tile([DIN, DH], bf16, tag="w1t")
            W2b_t = cpool.tile([CH, DOUT], bf16, tag="w2t")
            b1c_t = cpool.tile([CH, 1], f32, tag="b1")
            b2_t = cpool.tile([CH, DOUT], f32, tag="b2")
            ident = cpool.tile([CH, CH], f32, tag="ident")
            identb = cpool.tile([CH, CH], bf16, tag="identb")
            comb_t = cpool.tile([CH, NBUCK * (SHARD // 16)], i16, tag="combt")
            wt_b = [None] * NBUCK
            wt2_b = [None] * NBUCK
            nc.sync.dma_start(out=W1T_t[:], in_=W1Tb[:, :])
            nc.sync.dma_start(out=W2b_t[:DH, :], in_=W2Tb[:, :])
            nc.sync.dma_start(out=W2b_t[DH:2 * DH, :], in_=W2Tb[:, :])
            nc.sync.dma_start(out=b1c_t[:DH, :], in_=b1col[:, :])
            nc.sync.dma_start(out=b1c_t[DH:2 * DH, :], in_=b1col[:, :])
            nc.sync.dma_start(out=b2_t[:], in_=b2bc[:, :])
            nc.sync.dma_start(out=comb_t[:], in_=comb_idx[:, :])
            make_identity(nc, ident[:])
            make_identity(nc, identb[:])

            ZW = NCHUNK * DOUT // 4
            zt = cpool.tile([CH, ZW], bf16, tag="zt")
            nc.vector.memset(zt[:], 0.0)

            def zero_steps():
                steps = []

                def za(j):
                    nc.sync.dma_start(
                        out=acc1[:, j * ZW:(j + 1) * ZW], in_=zt[:])

                def zw(a0, h):
                    nc.sync.dma_start(
                        out=partial2[a0 * CH:(a0 + 1) * CH,
                                     h * ZW:(h + 1) * ZW],
                        in_=zt[:])

                for j in range(NCHUNK * DH // ZW):
                    steps.append(lambda j_=j: za(j_))
                for a0 in range(NC):
                    for h in range(4):
                        steps.append(lambda a_=a0, h_=h: zw(a_, h_))
                return steps

            deg_a = cpool.tile([CH, meta["NT"] * VS], f32, tag="dega")
            dinv_a = cpool.tile([CH, meta["NT"] * VS], f32, tag="dinva")
            deg_o = cpool.tile([CH, NCHUNK], f32, tag="dego")
            dinv_o = cpool.tile([CH, NCHUNK], f32, tag="dinvo")
            nc.vector.memset(deg_a[:], 0.0)
            nc.vector.memset(deg_o[:], 0.0)
            for (q0, g, K, off) in meta["degown_groups"]:
                m = g * K
                t = dpool.tile([CH, DEG_CAP], bf16, tag="degt")
                nc.sync.dma_start(
                    out=t[:, :m],
                    in_=wdeg_own[0, off:off + CH * m].rearrange("(p m) -> p m", p=CH))
                nc.vector.tensor_reduce(
                    out=deg_o[:, q0:q0 + g],
                    in_=t[:, :m].rearrange("p (g k) -> p g k", g=g),
                    axis=mybir.AxisListType.X, op=mybir.AluOpType.add)
            nc.vector.tensor_scalar_add(out=deg_o[:], in0=deg_o[:], scalar1=1.0)
            nc.scalar.sqrt(out=dinv_o[:], in_=deg_o[:])
            nc.vector.reciprocal(out=dinv_o[:], in_=dinv_o[:])

            Dh = DH
            HC = NCHUNK // 2
            AV = 2 * DH
            scat1_q = []
            acc1rows = acc1[:, :].rearrange("p (u f) -> (p u) f", f=AV)

            def scat1_step(rt_, g_, q0_, b_):
                par = q0_ // HC
                i0 = b_ * (SHARD // 16)
                nc.gpsimd.dma_scatter_add(
                    out_ap=acc1rows[:, par * DH:(par + 1) * DH],
                    in_ap=rt_.rearrange("p (g d) -> p g d", g=g_),
                    idxs_ap=comb_t[:, i0 + q0_ * 8:i0 + (q0_ + g_) * 8],
                    num_idxs=CH * g_, num_idxs_reg=CH * g_, elem_size=DH,
                    elem_step=AV, single_packet=False, queue_num=1)

            sm1 = {"rt": None, "q0": -1, "fill": 0, "b": -1}

            def flush_scat1():
                if sm1["fill"]:
                    scat1_q.append(
                        lambda rt_=sm1["rt"][:, :sm1["fill"] * DH],
                        g_=sm1["fill"], q0_=sm1["q0"], b_=sm1["b"]:
                        scat1_step(rt_, g_, q0_, b_))
                    sm1["rt"] = None
                    sm1["fill"] = 0
                    while len(scat1_q) > SCAT_LAG:
                        scat1_q.pop(0)()

            def build_steps(b):
                steps = []

                def deg_step(T0, g, K, off):
                    m = g * VS * K
                    t = dpool.tile([CH, DEG_CAP], bf16, tag="degt", name="degt")
                    nc.sync.dma_start(
                        out=t[:, :m],
                        in_=wdeg[0, off:off + CH * m].rearrange(
                            "(p m) -> p m", p=CH))
                    nc.vector.tensor_reduce(
                        out=deg_a[:, T0 * VS:(T0 + g) * VS],
                        in_=t[:, :m].rearrange("p (v k) -> p v k", k=K),
                        axis=mybir.AxisListType.X, op=mybir.AluOpType.add)

                def dinv_step():
                    dsl = slice(b * TPB * VS, (b + 1) * TPB * VS)
                    nc.vector.tensor_scalar_add(
                        out=deg_a[:, dsl], in0=deg_a[:, dsl], scalar1=1.0)
                    nc.scalar.sqrt(out=dinv_a[:, dsl], in_=deg_a[:, dsl])
                    nc.vector.reciprocal(out=dinv_a[:, dsl], in_=dinv_a[:, dsl])

                def tile_step(tt):
                    T = b * TPB + tt
                    r0 = tt * TSZ
                    xt = xtp.tile([CH, VS * DIN], bf16, tag="xt", name="xt")
                    nc.sync.dma_start(
                        out=xt[:],
                        in_=x_perm[T * TSZ:(T + 1) * TSZ, :].rearrange(
                            "(p s) f -> p (s f)", p=CH))
                    xf = xtp.tile([CH, VS * DIN], f32, tag="xf", name="xf")
                    nc.vector.tensor_tensor(
                        out=xf[:].rearrange("p (s f) -> p s f", s=VS),
                        in0=xt[:].rearrange("p (s f) -> p s f", s=VS),
                        in1=dinv_a[:, T * VS:(T + 1) * VS][:, :, None]
                        .to_broadcast([CH, VS, DIN]),
                        op=mybir.AluOpType.mult)
                    nc.sync.dma_start(
                        out=tables[b][r0:r0 + TSZ, :].rearrange(
                            "(p s) f -> p (s f)", p=CH),
                        in_=xf[:])

                def stage_step():
                    cols_b = meta["colsb"][b]
                    it = ipool.tile([CH, CBW * 8], i16, tag="it", name="it")
                    nc.sync.dma_start(
                        out=it[:, :cols_b * 8],
                        in_=ell_idx[:, BB[b] * 8:(BB[b] + cols_b) * 8])
                    wt = wpool.tile([CH, CB_MAX], f32, tag="wt", name="wt")
                    nc.sync.dma_start(out=wt[:, :cols_b],
                                      in_=ell_w[:, BB[b]:BB[b] + cols_b])
                    wt_b[b] = wt
                    it_b[b] = it

                steps.append(stage_step)
                for (T0, g, K, off) in meta["dega_groups"][b]:
                    steps.append(lambda a=T0, bg=g, cK=K, do=off: deg_step(a, bg, cK, do))
                steps.append(dinv_step)
                for tt in range(TPB):
                    steps.append(lambda t_=tt: tile_step(t_))
                return steps

            st2_b = [None] * NBUCK

            def stage2(b):
                cols_b = meta["cols2b"][b]
                it = ipool.tile([CH, CBW * 8], i16, tag="it", name="it")
                nc.sync.dma_start(out=it[:, :cols_b * 8],
                                  in_=ell_idx2[:, BB2[b] * 8:(BB2[b] + cols_b) * 8])
                wt2 = w2pool.tile([CH, CB2_MAX], f32, tag="wt2", name="wt2")
                nc.sync.dma_start(out=wt2[:, :cols_b],
                                  in_=ell_w2[:, BB2[b]:BB2[b] + cols_b])
                wt2_b[b] = wt2
                sc = cgpool.tile([CH, PB // 16], i16, tag="sc", name="sc")
                nc.sync.dma_start(out=sc[:],
                                  in_=scat_idx[:, b * (PB // 16):(b + 1) * (PB // 16)])
                st2_b[b] = (it, sc)

            def mm_group(q0, sg):
                asl = spool.tile([CH, MMG * Dh], bf16, tag="aslb")
                nc.sync.dma_start(out=asl[:, :sg * Dh],
                                  in_=acc1[:, q0 * Dh:(q0 + sg) * Dh])
                a2 = spool.tile([CH, MMG * Dh], f32, tag="a2")
                a_sl = a2[:, :sg * Dh]
                st = spool.tile([CH, MMG * Dh], bf16, tag="st")
                nc.sync.dma_start(out=st[:, :sg * Dh],
                                  in_=x_own[:, q0 * Dh:(q0 + sg) * Dh])
                stf = spool.tile([CH, MMG * Dh], f32, tag="stf")
                nc.vector.tensor_tensor(
                    out=stf[:, :sg * Dh].rearrange("p (j f) -> p j f", j=sg),
                    in0=st[:, :sg * Dh].rearrange("p (j f) -> p j f", j=sg),
                    in1=dinv_o[:, q0:q0 + sg][:, :, None]
                    .to_broadcast([CH, sg, Dh]),
                    op=mybir.AluOpType.mult)
                nc.vector.tensor_tensor(out=a_sl, in0=asl[:, :sg * Dh],
                                        in1=stf[:, :sg * Dh],
                                        op=mybir.AluOpType.add)
                nc.vector.tensor_tensor(
                    out=a_sl.rearrange("p (j f) -> p j f", j=sg),
                    in0=a_sl.rearrange("p (j f) -> p j f", j=sg),
                    in1=dinv_o[:, q0:q0 + sg][:, :, None]
                    .to_broadcast([CH, sg, Dh]),
                    op=mybir.AluOpType.mult)
                mm_ps = psum.tile([CH, MMG * DOUT], f32, tag="mmps", space="PSUM")
                a2b = spool.tile([CH, MMG * Dh], bf16, tag="a2b")
                nc.vector.tensor_copy(out=a2b[:, :sg * Dh],
                                      in_=a2[:, :sg * Dh])
                rls = []
                for jj0 in range(0, sg, 4):
                    bw = min(4, sg - jj0)
                    tp_ps = psumtp.tile([CH, 4 * CH], bf16, tag="tpps",
                                        space="PSUM")
                    for t in range(bw):
                        nc.tensor.transpose(
                            out=tp_ps[:Dh, t * CH:(t + 1) * CH],
                            in_=a2b[:, (jj0 + t) * Dh:(jj0 + t + 1) * Dh],
                            identity=identb[:])
                    tp_sb = tpool.tile([CH, 4 * CH], bf16, tag="tpsb")
                    nc.scalar.copy(out=tp_sb[:Dh, :bw * CH],
                                   in_=tp_ps[:Dh, :bw * CH])
                    h1_ps = psumtp.tile([CH, 4 * CH], f32, tag="h1ps",
                                        space="PSUM")
                    for t in range(bw):
                        nc.tensor.matmul(
                            out=h1_ps[:DH, t * CH:(t + 1) * CH],
                            lhsT=W1T_t[:DIN, :],
                            rhs=tp_sb[:Dh, t * CH:(t + 1) * CH],
                            start=True, stop=True)
                    rl_sb = tpool.tile([CH, 4 * CH], bf16, tag="rlsb")
                    nc.scalar.activation(
                        out=rl_sb[:DH, :bw * CH], in_=h1_ps[:DH, :bw * CH],
                        func=mybir.ActivationFunctionType.Relu,
                        bias=b1c_t[:DH, 0:1])
                    rls.append(rl_sb)
                for jj in range(sg):
                    nc.tensor.matmul(
                        out=mm_ps[:, jj * DOUT:(jj + 1) * DOUT],
                        lhsT=rls[jj // 4][:DH, (jj % 4) * CH:(jj % 4 + 1) * CH],
                        rhs=W2b_t[:DH, :],
                        start=True, stop=True)
                h = mpool.tile([CH, MMG * DOUT], f32, tag="hmm")
                nc.vector.tensor_tensor(
                    out=h[:, :sg * DOUT].rearrange("p (j f) -> p j f", j=sg),
                    in0=mm_ps[:, :sg * DOUT].rearrange("p (j f) -> p j f", j=sg),
                    in1=dinv_o[:, q0:q0 + sg][:, :, None]
                    .to_broadcast([CH, sg, DOUT]),
                    op=mybir.AluOpType.mult)
                nc.sync.dma_start(
                    out=t2shard[:, q0 * DH:(q0 + sg) * DH].rearrange(
                        "p (j f) -> p j f", j=sg)[:, :, :DOUT],
                    in_=h[:, :sg * DOUT].rearrange("p (j f) -> p j f", j=sg))

            it_b = [None] * NBUCK
            for s in build_steps(0):
                s()
            deferred = []
            zs_all = zero_steps()
            zdefer = zs_all[24:]
            for b in range(NBUCK):
                if b + 1 < NBUCK:
                    deferred = build_steps(b + 1)
                if b == 0:
                    deferred = deferred + zs_all[:24]
                it = it_b[b]
                for gi, (c0, cols, runs) in enumerate(meta["agg_groups"][b]):
                    cl = c0 - BB[b]
                    gd = gpool.tile([CH, TILE_COLS * Dh], f32, tag="gd")
                    nc.gpsimd.dma_gather(
                        out_ap=gd[:, :cols * Dh].rearrange("p (c d) -> p c d", c=cols),
                        in_ap=tables[b][:, :],
                        idxs_ap=it[:, cl * 8:(cl + cols) * 8],
                        num_idxs=CH * cols, num_idxs_reg=CH * cols, elem_size=Dh,
                        single_packet=False)
                    if len(runs) == 1 and runs[0][2] == 1:
                        kone1 = True
                        (q0, g, K, csg) = runs[0]
                    else:
                        kone1 = False
                        nc.vector.tensor_tensor(
                            out=gd[:, :cols * Dh].rearrange("p (c d) -> p c d", c=cols),
                            in0=gd[:, :cols * Dh].rearrange("p (c d) -> p c d", c=cols),
                            in1=wt_b[b][:, cl:cl + cols][:, :, None]
                            .to_broadcast([CH, cols, Dh]),
                            op=mybir.AluOpType.mult)
                    for (q0, g, K, csg) in runs:
                        off = csg - c0
                        if (sm1["fill"] == 0 or sm1["b"] != b
                                or q0 != sm1["q0"] + sm1["fill"]
                                or q0 // HC != sm1["q0"] // HC
                                or sm1["fill"] + g > SCMERGE1):
                            flush_scat1()
                            sm1.update(rt=rpool.tile([CH, SCMERGE1 * Dh],
                                                     bf16, tag="rt",
                                                     name="rt"),
                                       q0=q0, fill=0, b=b)
                        rt = sm1["rt"]
                        f0 = sm1["fill"]
                        rsl = rt[:, f0 * Dh:(f0 + g) * Dh]
                        if kone1:
                            nc.vector.tensor_tensor(
                                out=rsl.rearrange("p (c d) -> p c d", c=g),
                                in0=gd[:, off * Dh:(off + g) * Dh].rearrange(
                                    "p (c d) -> p c d", c=g),
                                in1=wt_b[b][:, cl + off:cl + off + g]
                                [:, :, None].to_broadcast([CH, g, Dh]),
                                op=mybir.AluOpType.mult)
                        else:
                            with nc.allow_low_precision("bf16 L1 acc"):
                                nc.vector.tensor_reduce(
                                    out=rsl.rearrange("p (g d) -> p g d", g=g),
                                    in_=gd[:, off * Dh:(off + g * K) * Dh]
                                    .rearrange("p (g k d) -> p g d k",
                                               g=g, k=K),
                                    axis=mybir.AxisListType.X,
                                    op=mybir.AluOpType.add)
                        sm1["fill"] += g
                    for _ in range(6):
                        if deferred:
                            deferred.pop(0)()
                while deferred:
                    deferred.pop(0)()
            flush_scat1()
            while scat1_q:
                scat1_q.pop(0)()

            for s_ in range(NS):
                mm_group(*mm_groups[s_])
                for _ in range(2):
                    if zdefer:
                        zdefer.pop(0)()
                if s_ == 0:
                    stage2(0)
                if s_ == 2:
                    stage2(1)

            QD = 4 * DOUT
            t2rows = t2shard[:, :].rearrange("p (u f) -> (p u) f", f=DH)
            scat_q = []

            def scat_step(rt_, g_, q0_, b_, sc_):
                pv = partial2[b_ * CPB * CH:(b_ + 1) * CPB * CH, :].rearrange(
                    "a (u f) -> (a u) f", f=QD)
                cls = q0_ // QCK
                nc.gpsimd.dma_scatter_add(
                    out_ap=pv[:, cls * DOUT:(cls + 1) * DOUT],
                    in_ap=rt_.rearrange("p (g d) -> p g d", g=g_),
                    idxs_ap=sc_[:, q0_ * 8:(q0_ + g_) * 8],
                    num_idxs=CH * g_, num_idxs_reg=CH * g_, elem_size=DOUT,
                    elem_step=QD,
                    single_packet=False, queue_num=1)

            sm = {"rt": None, "q0": -1, "fill": 0, "b": -1, "sc": None}

            def flush_scat():
                if sm["fill"]:
                    scat_q.append(
                        lambda rt_=sm["rt"][:, :sm["fill"] * DOUT],
                        g_=sm["fill"], q0_=sm["q0"], b_=sm["b"],
                        sc_=sm["sc"]: scat_step(rt_, g_, q0_, b_, sc_))
                    sm["rt"] = None
                    sm["fill"] = 0
                    while len(scat_q) > SCAT_LAG:
                        scat_q.pop(0)()

            for b in range(NBUCK):
                it, sc = st2_b[b]
                for g2i, (c0, cols, runs) in enumerate(meta["agg2_groups"][b]):
                    cl = c0 - BB2[b]
                    gd = gpool.tile([CH, TILE_COLS * DH], f32, tag="gd")
                    nc.gpsimd.dma_gather(
                        out_ap=gd[:, :cols * DH].rearrange("p (c d) -> p c d", c=cols),
                        in_ap=t2rows,
                        idxs_ap=it[:, cl * 8:(cl + cols) * 8],
                        num_idxs=CH * cols, num_idxs_reg=CH * cols, elem_size=DH,
                        single_packet=False)
                    kone = len(runs) == 1 and runs[0][2] == 1
                    if not kone:
                        nc.vector.tensor_tensor(
                            out=gd[:, :cols * DH].rearrange(
                                "p (c d) -> p c d", c=cols)[:, :, :DOUT],
                            in0=gd[:, :cols * DH].rearrange(
                                "p (c d) -> p c d", c=cols)[:, :, :DOUT],
                            in1=wt2_b[b][:, cl:cl + cols][:, :, None]
                            .to_broadcast([CH, cols, DOUT]),
                            op=mybir.AluOpType.mult)
                    for (q0, g, K, csg) in runs:
                        off = csg - c0
                        if (sm["fill"] == 0 or sm["b"] != b
                                or q0 != sm["q0"] + sm["fill"]
                                or q0 // QCK != sm["q0"] // QCK
                                or sm["fill"] + g > SCMERGE):
                            flush_scat()
                            sm.update(rt=r2pool.tile([CH, SCMERGE * DOUT],
                                                     bf16, tag="rt2b",
                                                     name="rtb"),
                                      q0=q0, fill=0, b=b, sc=sc)
                        rt = sm["rt"]
                        f0 = sm["fill"]
                        rsl = rt[:, f0 * DOUT:(f0 + g) * DOUT]
                        if kone:
                            nc.vector.tensor_tensor(
                                out=rsl.rearrange("p (g d) -> p g d", g=g),
                                in0=gd[:, off * DH:(off + g) * DH].rearrange(
                                    "p (g d) -> p g d", g=g)[:, :, :DOUT],
                                in1=wt2_b[b][:, cl + off:cl + off + g]
                                [:, :, None].to_broadcast([CH, g, DOUT]),
                                op=mybir.AluOpType.mult)
                        else:
                            with nc.allow_low_precision("bf16 L2 partials"):
                                nc.vector.tensor_reduce(
                                    out=rsl.rearrange("p (g d) -> p g d", g=g),
                                    in_=gd[:, off * DH:(off + g * K) * DH]
                                    .rearrange("p (g k d) -> p g d k",
                                               g=g, k=K)[:, :, :DOUT, :],
                                    axis=mybir.AxisListType.X,
                                    op=mybir.AluOpType.add)
                        sm["fill"] += g
                    if g2i == 0 and b + 1 < NBUCK:
                        stage2(b + 1)
            flush_scat()
            while scat_q:
                scat_q.pop(0)()

            nc.gpsimd.collective_compute(
                "ReduceScatter", mybir.AluOpType.add,
                replica_groups=[list(range(NC))],
                ins=[partial2[:, :].opt()],
                outs=[rs_out[:, :].opt()])

            TH = NCHUNK // 5
            t2bs = []
            for hh in range(5):
                t2b = cpool.tile([CH, TH * DOUT], f32, tag=f"t2b{hh}",
                                 name=f"t2b{hh}")
                nc.sync.dma_start(
                    out=t2b[:].rearrange("p (j f) -> p j f", j=TH),
                    in_=t2shard[:, hh * TH * DH:(hh + 1) * TH * DH].rearrange(
                        "p (j f) -> p j f", j=TH)[:, :, :DOUT])
                t2bs.append(t2b)
            for hh in range(5):
                csl = slice(hh * TH * DOUT, (hh + 1) * TH * DOUT)
                rsb = spool.tile([CH, TH * DOUT], bf16, tag="rsb")
                nc.sync.dma_start(out=rsb[:], in_=rs_out[:, csl])
                fin = mpool.tile([CH, TH * DOUT], f32, tag="fin")
                nc.vector.tensor_tensor(out=fin[:], in0=rsb[:],
                                        in1=t2bs[hh][:],
                                        op=mybir.AluOpType.add)
                nc.vector.tensor_tensor(
                    out=fin[:].rearrange("p (j f) -> p j f", j=TH),
                    in0=fin[:].rearrange("p (j f) -> p j f", j=TH),
                    in1=dinv_o[:, hh * TH:(hh + 1) * TH][:, :, None]
                    .to_broadcast([CH, TH, DOUT]),
                    op=mybir.AluOpType.mult)
                nc.vector.tensor_tensor(
                    out=fin[:].rearrange("p (j f) -> p j f", j=TH),
                    in0=fin[:].rearrange("p (j f) -> p j f", j=TH),
                    in1=b2_t[:, None, :].to_broadcast([CH, TH, DOUT]),
                    op=mybir.AluOpType.add)
                nc.sync.dma_start(out=out[:, csl], in_=fin[:])

    nc.compile()
    _split_multi_waits(nc)
    return nc


_CACHE = {}


def kernel(x, edge_index, edge_weights, W1, b1, W2, b2):
    x = np.asarray(x); edge_index = np.asarray(edge_index)
    edge_weights = np.asarray(edge_weights)
    W1 = np.asarray(W1); b1 = np.asarray(b1)
    W2 = np.asarray(W2); b2 = np.asarray(b2)

    meta, in_maps, tpos = _prep(x, edge_index, edge_weights, W1, b1, W2, b2)
    key = (x.shape, edge_index.shape, meta["TCOLS"], meta["TCOLS2"],
           meta["LDEG"], meta["LDEGO"], meta["agg_groups"], meta["agg2_groups"],
           meta["dega_groups"], meta["degown_groups"])
    if key not in _CACHE:
        _CACHE[key] = _build(meta)
    nc = _CACHE[key]
    res = run_bass_kernel_spmd(nc, in_maps, list(range(NC)))
    NCHUNK, DOUT, SHARD = meta["NCHUNK"], meta["DOUT"], meta["SHARD"]
    blocks = [res.results[c]["out"].reshape(CH, NCHUNK, DOUT).transpose(1, 0, 2)
              .reshape(SHARD, DOUT) for c in range(NC)]
    full = np.concatenate(blocks, axis=0)
    return full[tpos].astype(np.float32)

